# revision 19
# baseline (speedup 1.0000x reference)
"""BERT encoder layer on 8 TRN2 NeuronCores (Bass/Tile), data-parallel over batch.

Full inputs: hidden_states [16, 512, 1024], attention_mask [16, 512], weights.
Each core processes 2 batch items (1024 tokens). Weights are replicated; no
collectives. PSUM accumulation, residuals and LayerNorm run in fp32.

Precision: QKV + attention-context + attention-output matmuls run in fp8
e4m3 with DoubleRow perf mode (2 contraction subtiles per instruction, 2x
throughput); weights are host-scaled by 32 so fp8 sees ~N(0,1), and the
dequant folds into existing PSUM-evacuation ops. The FFN and the attention
scores (K=64, output-bound -- fp8 wouldn't help) run in fp16, which costs
the same as bf16 but leaves error budget for the fp8 stages. Simulated
end-to-end rel err ~8e-3 (QK_FP8=False) / ~1.6e-2 (True); gate is 2e-2.

Layout: activations flow feature-major ("T" suffix = [feature, token]); the
x transpose is done on the host (xt8 is already feature-major fp8).

Attention per head: scores are computed transposed [key, query]. exp runs
with a -4 bias so e = exp(s-4) fits fp8 range (scores max ~8.9 < 4+ln 240);
the shift cancels in the softmax ratio. The mask enters as exp(mask) folded
into v. v_aug rows are [32s(64) | 32v(64)]: the constant-32 columns make
ctx-matmul output rows 0:64 the (replicated) denominator at base partition
0 -- where the DVE reciprocal can read PSUM directly, no staging copy --
and rows 64:128 hold 32*ctx; the 32 cancels in num/den.

Schedule: the scalar engine's exp chain (~2.2us per attention iteration) is
the latency driver, so attention starts as soon as batch 0's Q/K/V are done
and every later PE stage is interleaved INTO the attention loops:
batch 0's iterations hide the jh=1 QKV matmuls, batch 1's hide the
attention-dense + LN1 + transposes + first FFN-intermediate chunks. Weight
DMAs are batched (one per matrix or half) to cut SWDGE issue cost.
"""

import numpy as np
import ml_dtypes

import concourse.bass as bass
import concourse.mybir as mybir
import concourse.tile as tile
from concourse import bacc
from concourse.bass_utils import run_bass_kernel_spmd
from concourse.masks import make_identity

F32 = mybir.dt.float32
BF16 = mybir.dt.bfloat16
FP16 = mybir.dt.float16
FP8 = mybir.dt.float8e4
AF = mybir.ActivationFunctionType
OP = mybir.AluOpType
DR = mybir.MatmulPerfMode.DoubleRow

B, S, D, H, F = 16, 512, 1024, 16, 4096
DH = D // H                      # 64
LN_EPS = 1e-12
NCORES = 8
BPC = B // NCORES                # 2 batch items per core
T = BPC * S                      # 1024 tokens per core
P = 128
DSUB = D // P                    # 8
TCH = T // P                     # 8 token chunks
SCH = S // P                     # 4 key chunks per batch item
NT = 512                         # matmul moving-dim tile (PSUM bank limit)
NR = 4                           # FFN rounds (interT ring-buffers 2 of them)
FSH = F // NR // P               # 8 Wi feature subtiles per round
VW = 2 * DH                      # 128: v_aug row = [32s(64) | 32v(64)]
EXP_SHIFT = 4.0                  # e = exp(s-4) keeps e < 240 (fp8 max)
WSC = 32.0                       # host premultiplier on fp8 weights
QK_FP8 = True                    # False -> q/k in fp16 (safer, +~20us)

QK_DT = FP8 if QK_FP8 else FP16
QK_WDT = FP8 if QK_FP8 else FP16
Q_SCALE = 1.0 / (np.sqrt(DH) * (WSC if QK_FP8 else 1.0))
K_SCALE = 1.0 / WSC if QK_FP8 else 1.0


class _Pool:
    """Manually-scoped tile pool (pools must close in LIFO stack order)."""

    def __init__(self, tc, name, bufs, space="SBUF"):
        self._cm = tc.tile_pool(name=name, bufs=bufs, space=space)
        self.pool = self._cm.__enter__()

    def tile(self, *a, **k):
        if "name" not in k:
            k["name"] = k.get("tag", "t")
        return self.pool.tile(*a, **k)

    def close(self):
        self._cm.__exit__(None, None, None)


def _load_bias_cols(nc, pool, dram_vec, n_sub, tag, scale=None):
    """[n_sub*P] DRAM vector -> [P, n_sub] SBUF (feature d -> [d%P, d//P])."""
    col = pool.tile([P, n_sub], F32, tag=tag)
    nc.scalar.dma_start(col[:], dram_vec.rearrange("(c p) -> p c", p=P))
    if scale is not None:
        nc.vector.tensor_scalar_mul(col[:], col[:], scale)
    return col


def _load_bcast(nc, pool, dram_vec, tag):
    """[D] DRAM vector -> [P, D] SBUF via one-row DMA + on-chip broadcast."""
    t = pool.tile([P, dram_vec.shape[0]], F32, tag=tag)
    nc.scalar.dma_start(out=t[0:1, :], in_=dram_vec)
    nc.gpsimd.partition_broadcast(t[:], t[0:1, :])
    return t


def build_bert_layer(tc):
    nc = tc.nc
    dt = nc.dram_tensor
    xt8_d = dt("xt8", [D, T], FP8, kind="ExternalInput")
    xf_d = dt("xf", [T, D], F32, kind="ExternalInput")
    mask_d = dt("mask", [BPC, S], F32, kind="ExternalInput")
    wq_d = dt("Wq", [D, D], QK_WDT, kind="ExternalInput")
    bq_d = dt("bq", [D], F32, kind="ExternalInput")
    wk_d = dt("Wk", [D, D], QK_WDT, kind="ExternalInput")
    bk_d = dt("bk", [D], F32, kind="ExternalInput")
    wv_d = dt("Wv", [D, D], FP8, kind="ExternalInput")
    bv_d = dt("bv", [D], F32, kind="ExternalInput")
    wo_d = dt("Wo", [D, D], FP8, kind="ExternalInput")
    g1_d = dt("ln1_g", [D], F32, kind="ExternalInput")
    b1o2_d = dt("b1o2", [D], F32, kind="ExternalInput")
    wi_d = dt("Wi", [D, F], FP16, kind="ExternalInput")
    bi_d = dt("bi", [F], F32, kind="ExternalInput")
    wo2_d = dt("Wo2", [F, D], FP16, kind="ExternalInput")
    g2_d = dt("ln2_g", [D], F32, kind="ExternalInput")
    b2_d = dt("ln2_b", [D], F32, kind="ExternalInput")
    y_d = dt("y", [T, D], BF16, kind="ExternalOutput")

    const = _Pool(tc, "const", 1)
    ident_f = const.tile([P, P], F32, tag="ident_f")
    make_identity(nc, ident_f)
    eps_col = const.tile([P, 1], F32, tag="eps")
    nc.vector.memset(eps_col, LN_EPS)
    nshift_col = const.tile([P, 1], F32, tag="nshift")
    nc.vector.memset(nshift_col, -EXP_SHIFT)
    bqs_col = _load_bias_cols(nc, const, bq_d.ap(), DSUB, "bqs",
                              scale=1.0 / np.sqrt(DH))
    bk_col = _load_bias_cols(nc, const, bk_d.ap(), DSUB, "bk")
    bi_col = _load_bias_cols(nc, const, bi_d.ap(), F // P, "bi")

    # PSUM pools are phase-local (created/closed around each stage); the
    # mutable PS dict lets the shared helpers pick up the current pools.
    PS = {}

    # Persistent activations (allocated up front; LIFO-safe across phases)
    p_fm = _Pool(tc, "fm", 1)
    ctxT = p_fm.tile([P, DSUB, T], FP8, tag="ctxT")
    p_atok = _Pool(tc, "atok", 1)
    a_tok = p_atok.tile([P, TCH, D], F32, tag="a_tok")
    p_aT = _Pool(tc, "aT", 1)
    aT = p_aT.tile([P, DSUB, T], FP16, tag="aT")
    # phase-3/4 support pools (opened early so closes stay LIFO)
    ph3w = _Pool(tc, "ph3w", 2)
    ph3x = _Pool(tc, "ph3x", 3)
    p_ln = _Pool(tc, "p_ln", 4)
    p_int = _Pool(tc, "inter", 1)
    interT = p_int.tile([P, 2 * FSH, T], FP16, tag="interT")
    ph5w = _Pool(tc, "ph5w", 2)

    # ---- Phase 0: DMAs -- x (host-pretransposed fp8) + batched weights ----
    p_qkv = _Pool(tc, "qkv", 1)
    xt = p_qkv.tile([P, DSUB, T], FP8, tag="xt")
    qT = p_qkv.tile([P, DSUB, T], QK_DT, tag="qT")
    kT = p_qkv.tile([P, DSUB, T], QK_DT, tag="kT")
    v_aug = p_qkv.tile([P, TCH, H, VW], FP8, tag="v_aug")
    wq_sb = p_qkv.tile([P, DSUB, D], QK_WDT, tag="wq_sb")
    wk_sb = p_qkv.tile([P, DSUB, D], QK_WDT, tag="wk_sb")

    xtr = xt8_d.ap().rearrange("(ds p) t -> p ds t", p=P)
    nc.sync.dma_start(xt[:, :, 0:NT], xtr[:, :, 0:NT])
    nc.gpsimd.dma_start(wq_sb[:], wq_d.ap().rearrange("(ks p) m -> p ks m", p=P))
    nc.scalar.dma_start(xt[:, :, NT:T], xtr[:, :, NT:T])
    nc.gpsimd.dma_start(wk_sb[:], wk_d.ap().rearrange("(ks p) m -> p ks m", p=P))
    ph1v = _Pool(tc, "ph1v", 2)
    wvr = wv_d.ap().rearrange("(ks p) m -> p ks m", p=P)
    wv_t = []
    for jh in range(2):
        wvt = ph1v.tile([P, DSUB, NT], FP8, tag="w_v")
        nc.sync.dma_start(wvt[:], wvr[:, :, jh * NT:(jh + 1) * NT])
        wv_t.append(wvt)

    def _emit_bcast_consts():
        bv_b = _load_bcast(nc, const, bv_d.ap(), "bv_b")
        # v rows carry the raw PSUM (32*v); bias must match that scale
        nc.vector.tensor_scalar_mul(bv_b[:], bv_b[:], WSC)
        g1_b = _load_bcast(nc, const, g1_d.ap(), "g1_b")
        g2_b = _load_bcast(nc, const, g2_d.ap(), "g2_b")
        b2_b = _load_bcast(nc, const, b2_d.ap(), "b2_b")
        # host precomputes b1o2 = ln1_b + bo2 (LN1 beta absorbs FFN bias)
        b1o2_b = _load_bcast(nc, const, b1o2_d.ap(), "b1o2")
        # mask[b, kt] -> [kt%P, b, kt//P]; em = exp(mask) folded into v rows
        mask_sb = const.tile([P, BPC, SCH], F32, tag="mask")
        for b in range(BPC):
            nc.scalar.dma_start(mask_sb[:, b, :],
                                mask_d.ap()[b].rearrange("(c p) -> p c", p=P))
        em_col = const.tile([P, BPC * SCH], F32, tag="em")
        nc.scalar.activation(em_col[:], mask_sb[:], AF.Exp)
        return bv_b, g1_b, g2_b, b2_b, b1o2_b, em_col

    bv_b, g1_b, g2_b, b2_b, b1o2_b, em_col = _emit_bcast_consts()
    nc.vector.memset(v_aug[:, :, :, 0:DH], WSC)
    ps1 = _Pool(tc, "ps1", 4, space="PSUM")
    PS["one"] = ps1

    # ---- Stage 1: QKV for batch 0 (jh=0); evac on scalar (idle here) ----
    def qk_proj(w_sb, dst, bias_col, scale, mo, jh, eng):
        ps = PS["one"].tile([P, 2, NT], F32, tag="big")[:, 0, :]
        wsl = w_sb[:, :, mo * P:(mo + 1) * P]
        mov = xt[:, :, jh * NT:(jh + 1) * NT]
        if QK_FP8:
            for i in range(DSUB // 2):
                nc.tensor.matmul(ps[:], wsl[:, 2 * i:2 * i + 2, :],
                                 mov[:, 2 * i:2 * i + 2, :],
                                 start=(i == 0), stop=(i == DSUB // 2 - 1),
                                 perf_mode=DR)
        else:
            for ks in range(DSUB):
                nc.tensor.matmul(ps[:], wsl[:, ks, :], mov[:, ks, :],
                                 start=(ks == 0), stop=(ks == DSUB - 1))
        eng.tensor_scalar(dst[:, mo, jh * NT:(jh + 1) * NT], ps[:],
                          scale, bias_col[:, mo:mo + 1], OP.mult, OP.add)

    def v_chunk(tc_i, jh):
        ps = PS["one"].tile([P, 2, NT], F32, tag="big")[:, 0, :]
        for i in range(DSUB // 2):
            nc.tensor.matmul(ps[:], xt[:, 2 * i:2 * i + 2,
                                       tc_i * P:(tc_i + 1) * P],
                             wv_t[jh][:, 2 * i:2 * i + 2, :],
                             start=(i == 0), stop=(i == DSUB // 2 - 1),
                             perf_mode=DR)
        nc.vector.tensor_tensor(
            v_aug[:, tc_i, jh * 8:(jh + 1) * 8, DH:VW], ps[:],
            bv_b[:, jh * NT:(jh + 1) * NT], OP.add)
        if jh == 1:
            # fold exp(mask) into v rows (incl. the 32s cols -> denominator)
            nc.vector.tensor_scalar_mul(v_aug[:, tc_i], v_aug[:, tc_i],
                                        em_col[:, tc_i:tc_i + 1])

    for mo in range(DSUB):
        qk_proj(wq_sb, qT, bqs_col, Q_SCALE, mo, 0, nc.vector)
        qk_proj(wk_sb, kT, bk_col, K_SCALE, mo, 0, nc.vector)
    for tc_i in range(SCH):
        v_chunk(tc_i, 0)
        v_chunk(tc_i, 1)

    # Wo tiles stream in during stage 2 (used from stage 3 on)
    wor = wo_d.ap().rearrange("(ks p) m -> p ks m", p=P)
    wo_t = []
    for jh in range(2):
        wt = ph3w.tile([P, DSUB, NT], FP8, tag="w_o")
        nc.gpsimd.dma_start(wt[:], wor[:, :, jh * NT:(jh + 1) * NT])
        wo_t.append(wt)

    # ---- Attention PSUM pools: scores 2x[P,2,NT] + cps 2x[P,NT] double-
    # buffered (so ctx(i+1) never waits on normalize(i)) + 1 unit tile ----
    ps1.close()
    ps_sc = _Pool(tc, "ps_sc", 2, space="PSUM")
    ps_cps = _Pool(tc, "ps_cps", 2, space="PSUM")
    ps_u = _Pool(tc, "ps_u", 1, space="PSUM")
    PS["one"] = ps_u
    PS["big"] = ps_u
    p_e = _Pool(tc, "p_e", 4)  # 2 attn iters in flight
    p_rec = _Pool(tc, "p_rec", 1)

    def attn_iter(b, h):
        hs, hr = h // 2, (h % 2) * DH
        bs = b * S
        s01 = ps_sc.tile([P, 2, NT], F32, tag="sc")
        s23 = ps_sc.tile([P, 2, NT], F32, tag="sc")
        for ci in range(2):
            nc.tensor.matmul(
                s01[:, ci, :],
                kT[hr:hr + DH, hs, bs + ci * P:bs + (ci + 1) * P],
                qT[hr:hr + DH, hs, bs:bs + S], start=True, stop=True)
        e01 = p_e.tile([P, 2, NT], FP8, tag="e")
        nc.scalar.activation(e01[:], s01[:], AF.Exp, bias=nshift_col[:])
        for ci in range(2):
            nc.tensor.matmul(
                s23[:, ci, :],
                kT[hr:hr + DH, hs, bs + (2 + ci) * P:bs + (3 + ci) * P],
                qT[hr:hr + DH, hs, bs:bs + S], start=True, stop=True)
        e23 = p_e.tile([P, 2, NT], FP8, tag="e")
        nc.scalar.activation(e23[:], s23[:], AF.Exp, bias=nshift_col[:])
        cps = ps_cps.tile([P, NT], F32, tag="cps")
        nc.tensor.matmul(cps[:], v_aug[:, b * SCH:b * SCH + 2, h, :], e01[:],
                         start=True, stop=False, perf_mode=DR)
        nc.tensor.matmul(cps[:], v_aug[:, b * SCH + 2:b * SCH + 4, h, :],
                         e23[:], start=False, stop=True, perf_mode=DR)
        # rows 0:64 = 32*denominator (from the constant-32 stationary cols,
        # base partition 0 so the DVE reciprocal reads PSUM directly),
        # rows 64:128 = 32*ctx; the 32 cancels in the ratio.
        rec = p_rec.tile([DH, NT], F32, tag="rec")
        nc.vector.reciprocal_approx_fast(rec[:], cps[0:DH, :])
        nc.vector.tensor_tensor(ctxT[hr:hr + DH, hs, bs:bs + S],
                                cps[DH:P, :], rec[:], OP.mult)

    xres_t = {}

    def ph3_x(tc_i):
        xres = ph3x.tile([P, D], F32, tag="xres")
        nc.sync.dma_start(xres[:], xf_d.ap()[tc_i * P:(tc_i + 1) * P, :])
        xres_t[tc_i] = xres

    ln_mv = {}

    def ph3_mm(tc_i):
        """Wo matmuls + residual + LN1 for one token chunk."""
        if tc_i not in xres_t:
            ph3_x(tc_i)
        xres = xres_t.pop(tc_i)
        ps = PS["big"].tile([P, 2, NT], F32, tag="big")
        for i in range(DSUB // 2):
            lhs = ctxT[:, 2 * i:2 * i + 2, tc_i * P:(tc_i + 1) * P]
            nc.tensor.matmul(ps[:, 0, :], lhs, wo_t[0][:, 2 * i:2 * i + 2, :],
                             start=(i == 0), stop=(i == DSUB // 2 - 1),
                             perf_mode=DR)
            nc.tensor.matmul(ps[:, 1, :], lhs, wo_t[1][:, 2 * i:2 * i + 2, :],
                             start=(i == 0), stop=(i == DSUB // 2 - 1),
                             perf_mode=DR)
        row = a_tok[:, tc_i, :]
        # PSUM holds 32*(ctx@Wo): scale by 1/32 while adding the residual
        # (xres = x + bo, folded on the host)
        nc.vector.scalar_tensor_tensor(row[:, 0:NT], ps[:, 0, :], 1.0 / WSC,
                                       xres[:, 0:NT], OP.mult, OP.add)
        nc.vector.scalar_tensor_tensor(row[:, NT:D], ps[:, 1, :], 1.0 / WSC,
                                       xres[:, NT:D], OP.mult, OP.add)
        st = p_ln.tile([P, 2, 6], F32, tag="ln1_st")
        nc.vector.bn_stats(st[:, 0, :], row[:, 0:NT])
        nc.vector.bn_stats(st[:, 1, :], row[:, NT:D])
        mv = p_ln.tile([P, 2], F32, tag="ln1_mv")
        nc.vector.bn_aggr(mv[:], st[:])
        nc.vector.scalar_tensor_tensor(row, row, mv[:, 0:1], g1_b[:],
                                       OP.subtract, OP.mult)
        ln_mv[tc_i] = mv

    def ph3_fin(tcis):
        """Batched istd, then apply *istd + beta per chunk."""
        n = len(tcis)
        var_c = p_ln.tile([P, n], F32, tag="ln1_var")
        for j, tci in enumerate(tcis):
            nc.vector.tensor_copy(var_c[:, j:j + 1], ln_mv[tci][:, 1:2])
        istd = p_ln.tile([P, n], F32, tag="ln1_istd")
        nc.scalar.activation(istd[:], var_c[:], AF.Sqrt, bias=eps_col[:],
                             scale=1.0)
        nc.vector.reciprocal_approx_fast(istd[:], istd[:])
        for j, tci in enumerate(tcis):
            nc.vector.scalar_tensor_tensor(a_tok[:, tci, :], a_tok[:, tci, :],
                                           istd[:, j:j + 1], b1o2_b[:],
                                           OP.mult, OP.add)

    def ph3_tr(tc_i):
        """PE-transpose one LN1'd chunk into feature-major aT (fp16)."""
        row = a_tok[:, tc_i, :]
        pst = PS["big"].tile([P, 2, NT], F32, tag="big")
        for g in range(2):
            for j in range(4):
                ds = g * 4 + j
                nc.tensor.transpose(pst[:, g, j * P:(j + 1) * P],
                                    row[:, ds * P:(ds + 1) * P], ident_f[:])
        for g in range(2):
            nc.vector.tensor_copy(
                aT[:, g * 4:(g + 1) * 4, tc_i * P:(tc_i + 1) * P], pst[:, g, :])

    # FFN weights stream in per-chunk on the (idle) sync DMA queue
    wir = wi_d.ap().rearrange("(ks p) m -> p ks m", p=P)
    wo2r = wo2_d.ap().rearrange("(ks p) m -> p ks m", p=P)

    def ffn_inter_half(fs, jh):
        """inter[:, fs, jh half] = gelu(aT @ Wi[:, fs]) -- fp16, 8 matmuls."""
        wt = ph5w.tile([P, DSUB, P], FP16, tag="w_i")
        nc.sync.dma_start(wt[:], wir[:, :, fs * P:(fs + 1) * P])
        ps = PS["one"].tile([P, 2, NT], F32, tag="big")[:, 0, :]
        for ks in range(DSUB):
            nc.tensor.matmul(ps[:], wt[:, ks, :],
                             aT[:, ks, jh * NT:(jh + 1) * NT],
                             start=(ks == 0), stop=(ks == DSUB - 1))
        nc.scalar.activation(interT[:, fs, jh * NT:(jh + 1) * NT], ps[:],
                             AF.Gelu, bias=bi_col[:, fs:fs + 1], scale=1.0)

    # ---- Stage 2: attention b0, hiding QKV jh=1 + v chunks 4..7 ----
    units = []
    for mo in range(DSUB):
        units.append(lambda mo=mo: qk_proj(wq_sb, qT, bqs_col, Q_SCALE,
                                           mo, 1, nc.vector))
        units.append(lambda mo=mo: qk_proj(wk_sb, kT, bk_col, K_SCALE,
                                           mo, 1, nc.vector))
    for tc_i in range(SCH, TCH):
        units.append(lambda t=tc_i: v_chunk(t, 0))
        units.append(lambda t=tc_i: v_chunk(t, 1))

    for h in range(H):
        attn_iter(0, h)
        units.pop(0)()
        if h % 2 == 0:
            units.pop(0)()
    while units:
        units.pop(0)()

    # ---- Stage 3: attention b1, hiding Wo/LN1 (b0) + transposes + FFN ----
    ph3_x(0)
    ph3_x(1)
    for h in range(H):
        attn_iter(1, h)
        if h < 4:
            ph3_mm(h)
            if h < 2:
                ph3_x(h + 2)
        elif h == 4:
            ph3_fin([0, 1, 2, 3])
        elif h <= 8:
            ph3_tr(h - 5)
        else:
            ffn_inter_half(h - 9, 0)
    p_rec.close()
    p_e.close()
    ps_u.close()
    ps_cps.close()
    ps_sc.close()
    ph1v.close()
    p_qkv.close()
    # stage-4-only pools (created after the qkv pool frees its 56 kb;
    # they close first at the end, so LIFO still holds)
    ph5 = _Pool(tc, "ph5", 4)
    p_y = _Pool(tc, "p_y", 2)
    p_ln2 = _Pool(tc, "p_ln2", 4)
    ps_b4 = _Pool(tc, "ps_b4", 3, space="PSUM")
    ps_14 = _Pool(tc, "ps_14", 1, space="PSUM")
    PS["big"] = ps_b4
    PS["one"] = ps_14

    # ---- Stage 4: LN1 (b1) + rest of FFN round 0, then rounds 1..3 with
    # inter(k+1) woven between ffn2(k) chunks (interT ring halves) ----
    for fs in range(7, FSH):
        ffn_inter_half(fs, 0)
    ph3_x(5)
    ph3_mm(4)
    ph3_x(6)
    ph3_mm(5)
    ph3_x(7)
    ph3_mm(6)
    ph3_mm(7)
    ph3_fin([4, 5, 6, 7])
    for tci in range(4, TCH):
        ph3_tr(tci)
    for fs in range(FSH):
        ffn_inter_half(fs, 1)

    def inter_round(k, fs):
        """Both-half inter unit for rounds >= 1; writes ring half k%2."""
        fchunk = k * FSH + fs
        wt = ph5w.tile([P, DSUB, P], FP16, tag="w_i")
        nc.sync.dma_start(wt[:], wir[:, :, fchunk * P:(fchunk + 1) * P])
        ps = ps_b4.tile([P, 2, NT], F32, tag="big")
        for ks in range(DSUB):
            nc.tensor.matmul(ps[:, 0, :], wt[:, ks, :], aT[:, ks, 0:NT],
                             start=(ks == 0), stop=(ks == DSUB - 1))
            nc.tensor.matmul(ps[:, 1, :], wt[:, ks, :], aT[:, ks, NT:T],
                             start=(ks == 0), stop=(ks == DSUB - 1))
        nc.scalar.activation(interT[:, (k % 2) * FSH + fs, :], ps[:], AF.Gelu,
                             bias=bi_col[:, fchunk:fchunk + 1], scale=1.0)

    w2r_t = {}

    def load_w2(k):
        tl = []
        for jh in range(2):
            wt2 = ph5.tile([P, FSH, NT], FP16, tag="w_o2")
            nc.gpsimd.dma_start(
                wt2[:], wo2r[:, k * FSH:(k + 1) * FSH, jh * NT:(jh + 1) * NT])
            tl.append(wt2)
        w2r_t[k] = tl

    def ffn2_chunk(k, tc_i):
        w2 = w2r_t[k]
        rb = (k % 2) * FSH
        ps = ps_b4.tile([P, 2, NT], F32, tag="big")
        for ks in range(FSH):
            lhs = interT[:, rb + ks, tc_i * P:(tc_i + 1) * P]
            nc.tensor.matmul(ps[:, 0, :], lhs, w2[0][:, ks, :],
                             start=(ks == 0), stop=(ks == FSH - 1))
            nc.tensor.matmul(ps[:, 1, :], lhs, w2[1][:, ks, :],
                             start=(ks == 0), stop=(ks == FSH - 1))
        row = a_tok[:, tc_i, :]
        nc.vector.tensor_tensor(row[:, 0:NT], row[:, 0:NT], ps[:, 0, :],
                                OP.add)
        nc.vector.tensor_tensor(row[:, NT:D], row[:, NT:D], ps[:, 1, :],
                                OP.add)
        if k == NR - 1:
            st = p_ln2.tile([P, 2, 6], F32, tag="ln2_st")
            nc.vector.bn_stats(st[:, 0, :], row[:, 0:NT])
            nc.vector.bn_stats(st[:, 1, :], row[:, NT:D])
            mv = p_ln2.tile([P, 2], F32, tag="ln2_mv")
            nc.vector.bn_aggr(mv[:], st[:])
            istd = p_ln2.tile([P, 1], F32, tag="ln2_istd")
            nc.scalar.activation(istd[:], mv[:, 1:2], AF.Sqrt,
                                 bias=eps_col[:], scale=1.0)
            nc.vector.reciprocal_approx_fast(istd[:], istd[:])
            yrow = p_y.tile([P, D], BF16, tag="yrow")
            nc.vector.scalar_tensor_tensor(yrow[:], row, mv[:, 0:1],
                                           g2_b[:], OP.subtract, OP.mult)
            nc.vector.scalar_tensor_tensor(yrow[:], yrow[:], istd[:],
                                           b2_b[:], OP.mult, OP.add)
            oeng = nc.sync if tc_i % 2 == 0 else nc.scalar
            oeng.dma_start(y_d.ap()[tc_i * P:(tc_i + 1) * P, :], yrow[:])

    load_w2(0)
    load_w2(1)
    for k in range(NR):
        for tc_i in range(TCH):
            ffn2_chunk(k, tc_i)
            if k + 1 < NR:
                inter_round(k + 1, tc_i)
        if k + 2 < NR:
            load_w2(k + 2)
    ps_14.close()
    ps_b4.close()
    p_ln2.close()
    p_y.close()
    ph5.close()

    ph5w.close()
    p_int.close()
    p_ln.close()
    ph3x.close()
    ph3w.close()
    p_aT.close()
    p_atok.close()
    p_fm.close()
    const.close()


def build_nc():
    nc = bacc.Bacc("TRN2", num_devices=NCORES)
    with tile.TileContext(nc) as tc:
        build_bert_layer(tc)
    nc.compile()
    return nc


_CACHE = {}


def make_in_maps(hidden_states, attention_mask, Wq, bq, Wk, bk, Wv, bv, Wo, bo,
                 ln1_g, ln1_b, Wi, bi, Wo2, bo2, ln2_g, ln2_b):
    f8 = ml_dtypes.float8_e4m3

    def w8(w, s):
        return np.asarray(np.asarray(w, np.float32) * s, f8)

    def wqk(w):
        return w8(w, WSC) if QK_FP8 else np.asarray(w, np.float16)

    common = {
        "Wq": wqk(Wq), "bq": np.asarray(bq, np.float32),
        "Wk": wqk(Wk), "bk": np.asarray(bk, np.float32),
        "Wv": w8(Wv, WSC), "bv": np.asarray(bv, np.float32),
        "Wo": w8(Wo, WSC),
        "ln1_g": np.asarray(ln1_g, np.float32),
        "b1o2": np.asarray(ln1_b, np.float32) + np.asarray(bo2, np.float32),
        "Wi": np.asarray(Wi, np.float16), "bi": np.asarray(bi, np.float32),
        "Wo2": np.asarray(Wo2, np.float16),
        "ln2_g": np.asarray(ln2_g, np.float32), "ln2_b": np.asarray(ln2_b, np.float32),
    }
    x = np.asarray(hidden_states, np.float32).reshape(B, S, D)
    m = np.asarray(attention_mask, np.float32).reshape(B, S)
    in_maps = []
    for c in range(NCORES):
        xc = np.ascontiguousarray(x[c * BPC:(c + 1) * BPC].reshape(T, D))
        in_maps.append({
            "xt8": np.asarray(np.ascontiguousarray(xc.T), f8),
            "xf": xc + np.asarray(bo, np.float32),
            "mask": np.ascontiguousarray(m[c * BPC:(c + 1) * BPC]),
            **common,
        })
    return in_maps


def kernel(**inputs) -> np.ndarray:
    if "nc" not in _CACHE:
        _CACHE["nc"] = build_nc()
    nc = _CACHE["nc"]
    in_maps = make_in_maps(**inputs)
    res = run_bass_kernel_spmd(nc, in_maps, core_ids=list(range(NCORES)))
    out = np.concatenate([np.asarray(res.results[c]["y"], np.float32)
                          for c in range(NCORES)], axis=0)
    return out.reshape(B, S, D)
